# revision 8
# baseline (speedup 1.0000x reference)
"""Causal multi-head attention on 8 Trainium2 NeuronCores — v3 (fp8 DoubleRow).

Sharding: core c -> (batch g = c // 4, head-group p = c % 4, heads 4p..4p+3).

v3 moves the Q/K/V projections and the QK^T scores to fp8e4m3 DoubleRow
matmuls (0.5 PE cycles per moving column vs 1.0 for bf16):

- Inputs arrive from host as fp8 hi/lo pairs (x8 = fp8(x), xr8 = fp8(x - x8))
  laid out [128, hl, blk, kcp, t, 512] so each DoubleRow matmul contracts
  2x128 input features per instruction. Projections compute the 3-term
  correction x8@W8 + xr8@W8 + x8@W8r (error ~0.1%), 12 DR matmuls per
  [128, 512] output tile vs 16 bf16 matmuls.
- Weights are host-scaled by 32 (w' = 32w) so fp8 quantization stays clear
  of the subnormal floor; the 32^2 factor on scores is folded into the exp
  scale (2^-13), and the 32 on V is cancelled by a 32.0 ones-column that
  also produces the softmax denominators.
- Q/K land directly in fp8 [128, pt, 2, S] with a zeroed DoubleRow sub-plane
  (the dk=64 contraction is packed as 64 real + 64 zero rows), so scores run
  DoubleRow at 0.5 cyc/col with no extra layout work. AV + O-proj stay bf16
  (probs/ctx quantization would blow the 2e-2 error budget).

Scheduling: projections stream through 2 PSUM banks and overlap the early
score/exp waves (Act starts ~10us in); AV opens after proj+V release their
banks (~27us) and catches up from the 19-deep probs buffer. Output is
ReduceScattered per 512-row wave (4 collectives) so only the last wave's
RS is exposed at the tail. PE warm-up dummies hold the pstate ramp.
"""

import numpy as np

B, S, D, H = 2, 2048, 1024, 16
DK = D // H
N_CORES = 8
FPC = 256  # features (head dims) per core
SCL = 32.0  # host-side weight scale

_CACHE = {}


def _build_nc():
    import concourse.mybir as mybir
    import concourse.tile as tile
    from concourse import bacc

    F32 = mybir.dt.float32
    BF16 = mybir.dt.bfloat16
    FP8 = mybir.dt.float8e4
    Exp = mybir.ActivationFunctionType.Exp
    DR = mybir.MatmulPerfMode.DoubleRow

    nc = bacc.Bacc("TRN2", target_bir_lowering=False, debug=False, num_devices=8)

    # weights: [p, hl, kcp, t, feat] fp8 (hi/lo), value = fp8(32 * w.T)
    cWQ = nc.dram_tensor("cWQ", [128, 2, 4, 2, 256], FP8, kind="ExternalInput")
    cWK = nc.dram_tensor("cWK", [128, 2, 4, 2, 256], FP8, kind="ExternalInput")
    cWV = nc.dram_tensor("cWV", [128, 2, 4, 2, 256], FP8, kind="ExternalInput")
    cWO = nc.dram_tensor("cWO", [128, 2, 1024], BF16, kind="ExternalInput")
    cMASK = nc.dram_tensor("cMASK", [128, 512], BF16, kind="ExternalInput")
    cF = nc.dram_tensor("cF", [128, 260], F32, kind="ExternalInput")
    # activations: [p, hl, blk, kcp, t, c] fp8 (x[256*kcp+128*t+p, 512*blk+c])
    xq8 = nc.dram_tensor("xq8", [128, 2, 4, 4, 2, 512], FP8, kind="ExternalInput")
    xk8 = nc.dram_tensor("xk8", [128, 2, 4, 4, 2, 512], FP8, kind="ExternalInput")
    xv8 = nc.dram_tensor("xv8", [128, 2, 4, 4, 2, 512], FP8, kind="ExternalInput")
    out = nc.dram_tensor("out", [512, D], BF16, kind="ExternalOutput")

    with tile.TileContext(nc) as tc:
        with (
            tc.tile_pool(name="consts", bufs=1) as consts,
            tc.tile_pool(name="persist", bufs=1) as persist,
            tc.tile_pool(name="xin", bufs=6) as xin,
            tc.tile_pool(name="prp", bufs=1) as prp,
            tc.tile_pool(name="small", bufs=3) as small,
            tc.tile_pool(name="oout", bufs=6) as oout,
            tc.tile_pool(name="dram", bufs=1, space="DRAM") as dram,
        ):
            # ---------------- constants ----------------
            cWQ_s = consts.tile([128, 2, 4, 2, 256], FP8, tag="cWQ", name="cWQ_s")
            cWK_s = consts.tile([128, 2, 4, 2, 256], FP8, tag="cWK", name="cWK_s")
            cWV_s = consts.tile([128, 2, 4, 2, 256], FP8, tag="cWV", name="cWV_s")
            cWO_s = consts.tile([128, 2, 1024], BF16, tag="cWO", name="cWO_s")
            mask4_s = consts.tile([128, 4, 128], BF16, tag="mask", name="mask4_s")
            cF_s = consts.tile([128, 260], F32, tag="cF", name="cF_s")
            warm_s = consts.tile([128, 512], BF16, tag="warm", name="warm_s")
            bq_s = cF_s[:, 0:2]
            bk_s = cF_s[:, 2:4]
            bvt_s = cF_s[:, 4:260].rearrange("p (h x) -> p h x", h=4)

            # ---------------- persistent activations ----------------
            # q/k in fp8, [p, pt, j, col]: j=0 data, j=1 zeros (DoubleRow pad)
            qT8 = persist.tile([128, 2, 2, S], FP8, tag="qT8", name="qT8")
            kT8 = persist.tile([128, 2, 2, S], FP8, tag="kT8", name="kT8")
            xv8_s = persist.tile([128, 2, 4, 4, 2, 512], FP8, tag="xv8", name="xv8_s")
            v_s = persist.tile([128, 16, 4, 65], BF16, tag="v", name="v_s")
            ctxT_s = persist.tile([128, 2, S], BF16, tag="ctxT", name="ctxT_s")

            rs_in = [dram.tile([512, D], BF16, name=f"rs_in{w}") for w in range(4)]
            rs_out = [dram.tile([128, D], BF16, name=f"rs_out{w}") for w in range(4)]

            # ---------------- t=0 setup ----------------
            # softmax denominator columns carry the 32x V scale
            for h in range(4):
                nc.gpsimd.memset(v_s[:, :, h, 64:65], SCL)
            # PE warm-up stationary/moving data
            nc.vector.memset(warm_s[:], 0.0)

            def zero_planes(b):
                # DoubleRow zero sub-planes, per 512-col block (DVE, emitted
                # just ahead of the proj blocks that fill the data plane)
                nc.vector.memset(kT8[:, :, 1, 512 * b : 512 * (b + 1)], 0.0)
                nc.vector.memset(qT8[:, :, 1, 512 * b : 512 * (b + 1)], 0.0)

            zero_planes(0)
            # warm the Exp table so LoadActFuncSet is off the critical path
            wexp = small.tile([1, 8], F32, tag="wexp", bufs=1, name="wexp")
            nc.vector.memset(wexp[:], 0.0)
            nc.scalar.activation(out=wexp[:], in_=wexp[:], func=Exp)

            # ---------------- input DMA stream (SP queue order) ----------------
            xq_t = {}  # (hl, qb) -> [128, 4, 2, 512] tile
            xk_t = {}

            def dma_qk(tag, src, store, hl, b):
                t = xin.tile([128, 4, 2, 512], FP8, tag="x", name=f"{tag}{hl}{b}")
                nc.sync.dma_start(t[:], src[:, hl, b])
                store[(hl, b)] = t

            nc.sync.dma_start(cF_s[:], cF.ap())
            nc.sync.dma_start(cWK_s[:], cWK.ap())
            nc.sync.dma_start(cWQ_s[:], cWQ.ap())
            for hl in range(2):
                dma_qk("xk", xk8, xk_t, hl, 0)
            for hl in range(2):
                dma_qk("xq", xq8, xq_t, hl, 0)
            nc.sync.dma_start(cWV_s[:], cWV.ap())
            for hl in range(2):
                nc.sync.dma_start(xv8_s[:, hl, 0], xv8[:, hl, 0])
            for b in range(1, 4):
                for hl in range(2):
                    dma_qk("xk", xk8, xk_t, hl, b)
                for hl in range(2):
                    dma_qk("xq", xq8, xq_t, hl, b)
                for hl in range(2):
                    nc.sync.dma_start(xv8_s[:, hl, b], xv8[:, hl, b])
            nc.sync.dma_start(cWO_s[:], cWO.ap())
            nc.sync.dma_start(
                mask4_s[:], cMASK.ap().rearrange("p (h x) -> p h x", h=4)
            )

            # ---------------- PSUM pools (manual phase sequencing) ----------
            psS = tc.alloc_tile_pool(name="psS", bufs=2, space="PSUM")  # 4 banks
            psV = tc.alloc_tile_pool(name="psV", bufs=2, space="PSUM")  # 2 banks
            psP = tc.alloc_tile_pool(name="psP", bufs=2, space="PSUM")  # 2 banks

            # PE pstate warm-up: dummy matmuls until the first real weights land
            for i in range(10):
                wps = psP.tile([128, 512], F32, tag="pp", name=f"warmmm{i}")
                nc.tensor.matmul(
                    wps[:], warm_s[:, 0:128], warm_s[:], start=True, stop=True
                )

            # ---------------- unit emitters ----------------
            def proj_block(w_s, x_t, b_s, outT, pt, b):
                """3-term fp8 DoubleRow projection of one [128, 512] tile:
                x8@W8 + xr8@W8 + x8@W8r, then bias-add writing fp8 qT8/kT8."""
                ps = psP.tile([128, 512], F32, tag="pp", name=f"pj{pt}{b}")
                terms = [(0, 0), (1, 0), (0, 1)]  # (x hl, w hl)
                n = 0
                for kcp in range(4):
                    for xh, wh in terms:
                        nc.tensor.matmul(
                            ps[:],
                            w_s[:, wh, kcp, :, 128 * pt : 128 * (pt + 1)],
                            x_t[(xh, b)][:, kcp],
                            start=(n == 0),
                            stop=(n == 11),
                            perf_mode=DR,
                        )
                        n += 1
                nc.vector.tensor_scalar_add(
                    outT[:, pt, 0, 512 * b : 512 * (b + 1)],
                    ps[:],
                    b_s[:, pt : pt + 1],
                )

            def v_unit(st):
                """V projection of one 128-kpos strip, 3-term fp8 DoubleRow,
                output in natural [kpos, feat] layout scaled by 32."""
                vb, i = st // 4, st % 4
                pv = psV.tile([128, 256], F32, tag="pv", name=f"pv{st}")
                n = 0
                for kcp in range(4):
                    for xh, wh in [(0, 0), (0, 1), (1, 0)]:
                        nc.tensor.matmul(
                            pv[:],
                            xv8_s[:, xh, vb, kcp, :, 128 * i : 128 * (i + 1)],
                            cWV_s[:, wh, kcp],
                            start=(n == 0),
                            stop=(n == 11),
                            perf_mode=DR,
                        )
                        n += 1
                nc.vector.tensor_add(
                    v_s[:, st, :, 0:64],
                    pv[:].rearrange("p (h x) -> p h x", x=64),
                    bvt_s,
                )

            pr_t = {}  # (w, ki) -> probs tile [128, 4, 512] bf16
            ctx_t = {}  # (w, h) -> ctx psum strip [65, 512] f32

            def sc_unit(w, ki):
                """fp8 DoubleRow scores + exp (+ diag mask) for (wave, ktile)."""
                qoff = 128 * max(ki - 4 * w, 0)
                wdt = 512 - qoff
                pr = prp.tile(
                    [128, 4, 512], BF16, tag="pr", bufs=19, name=f"pr_{w}_{ki}"
                )
                pr_t[(w, ki)] = pr
                for hp in range(2):
                    sc = psS.tile([128, 2, 512], F32, tag="sc", name="sc")
                    for j in range(2):
                        h = 2 * hp + j
                        r, pt = 64 * (h % 2), h // 2
                        nc.tensor.matmul(
                            sc[:, j, 0:wdt],
                            kT8[r : r + 64, pt, :, 128 * ki : 128 * (ki + 1)],
                            qT8[r : r + 64, pt, :, 512 * w + qoff : 512 * (w + 1)],
                            start=True,
                            stop=True,
                            perf_mode=DR,
                        )
                    nc.scalar.activation(
                        out=pr[:, 2 * hp : 2 * hp + 2, qoff:512],
                        in_=sc[:, :, 0:wdt],
                        func=Exp,
                        scale=2.0 ** -13,  # 1/8 softmax scale / 32^2 weight scale
                    )
                if ki >= 4 * w:  # diag ktile: mask upper triangle in place
                    nc.vector.tensor_mul(
                        pr[:, :, qoff : qoff + 128],
                        pr[:, :, qoff : qoff + 128],
                        mask4_s,
                    )

            psC_pool = [None]

            def av_open(w):
                for h in range(4):
                    ctx_t[(w, h)] = psC_pool[0].tile(
                        [65, 512], F32, tag="ctx", name=f"ctx{w}{h}"
                    )

            def av_mm(w, ki):
                """ctx^T[dk+1, q] += V_aug_h^T @ probs per head (bf16)."""
                qoff = 128 * max(ki - 4 * w, 0)
                last = 4 * w + 3
                for h in range(4):
                    nc.tensor.matmul(
                        ctx_t[(w, h)][:, qoff:512],
                        v_s[:, ki, h, :],
                        pr_t[(w, ki)][:, h, qoff:512],
                        start=(ki == 0),
                        stop=(ki == last),
                        skip_group_check=True,
                    )

            def av_fin(w, h):
                """normalize ctx^T by the denominators in partition 64."""
                r, pt = 64 * (h % 2), h // 2
                ctx = ctx_t[(w, h)]
                recip = small.tile([1, 512], F32, tag="recip", name="recip")
                nc.vector.reciprocal(recip[:], ctx[64:65, :])
                rbc = small.tile([64, 512], F32, tag="rbc", name="rbc")
                nc.gpsimd.partition_broadcast(rbc[:], recip[:])
                nc.vector.tensor_mul(
                    ctxT_s[r : r + 64, pt, 512 * w : 512 * (w + 1)],
                    ctx[0:64, :],
                    rbc[:],
                )

            def po_unit(w, u):
                qt, nb = u // 2, u % 2
                st = 4 * w + qt
                po = psC_pool[0].tile([128, 512], F32, tag="ctx", name="po")
                for fc in range(2):
                    nc.tensor.matmul(
                        po[:],
                        ctxT_s[:, fc, 128 * st : 128 * (st + 1)],
                        cWO_s[:, fc, 512 * nb : 512 * (nb + 1)],
                        start=(fc == 0),
                        stop=(fc == 1),
                    )
                ot = oout.tile([128, 512], BF16, tag="ot", name="ot")
                nc.vector.tensor_copy(ot[:], po[:])
                nc.sync.dma_start(
                    rs_in[w][128 * qt : 128 * (qt + 1), 512 * nb : 512 * (nb + 1)],
                    ot[:],
                )

            def rs_unit(w):
                import concourse.mybir as mybir_mod

                nc.gpsimd.collective_compute(
                    "ReduceScatter",
                    mybir_mod.AluOpType.add,
                    replica_groups=[[0, 1, 2, 3], [4, 5, 6, 7]],
                    ins=[rs_in[w].opt()],
                    outs=[rs_out[w].opt()],
                )

            # ---------------- static emission schedule ----------------
            def projK(pt, b):
                proj_block(cWK_s, xk_t, bk_s, kT8, pt, b)

            def projQ(pt, b):
                proj_block(cWQ_s, xq_t, bq_s, qT8, pt, b)

            # phase P: projections + V stream overlapped with waves 0-2 scores
            projK(0, 0); projK(1, 0); projQ(0, 0); projQ(1, 0)
            sc_unit(0, 0); sc_unit(0, 1)
            v_unit(0); v_unit(1)
            sc_unit(0, 2)
            v_unit(2); v_unit(3)
            sc_unit(0, 3)
            zero_planes(1)
            projK(0, 1); projK(1, 1); projQ(0, 1); projQ(1, 1)
            sc_unit(1, 0); sc_unit(1, 1)
            v_unit(4); v_unit(5)
            sc_unit(1, 2); sc_unit(1, 3)
            zero_planes(2)
            projK(0, 2); projK(1, 2); projQ(0, 2); projQ(1, 2)
            sc_unit(1, 4); sc_unit(1, 5)
            v_unit(6); v_unit(7); v_unit(8); v_unit(9)
            sc_unit(1, 6); sc_unit(1, 7)
            zero_planes(3)
            projK(0, 3); projK(1, 3); projQ(0, 3); projQ(1, 3)
            sc_unit(2, 0); sc_unit(2, 1)
            v_unit(10); v_unit(11); v_unit(12); v_unit(13)
            sc_unit(2, 2)
            v_unit(14); v_unit(15)

            # free proj+V banks, open the AV/oproj pool
            psP.release()
            psV.release()
            psC = tc.alloc_tile_pool(name="psC", bufs=4, space="PSUM")  # 4 banks
            psC_pool[0] = psC

            # wave 0 AV catch-up
            av_open(0)
            for ki in range(4):
                av_mm(0, ki)
            for h in range(4):
                av_fin(0, h)
            sc_unit(2, 3); sc_unit(2, 4)
            for u in range(8):
                po_unit(0, u)
            rs_unit(0)
            sc_unit(2, 5)
            # wave 1 AV
            av_open(1)
            for ki in range(4):
                av_mm(1, ki)
            sc_unit(2, 6)
            for ki in range(4, 8):
                av_mm(1, ki)
            for h in range(4):
                av_fin(1, h)
            sc_unit(2, 7)
            for u in range(4):
                po_unit(1, u)
            sc_unit(2, 8)
            for u in range(4, 8):
                po_unit(1, u)
            rs_unit(1)
            sc_unit(2, 9)
            # wave 2 AV
            av_open(2)
            for ki in range(6):
                av_mm(2, ki)
            sc_unit(2, 10)
            for ki in range(6, 10):
                av_mm(2, ki)
            sc_unit(2, 11)
            sc_unit(3, 0)
            av_mm(2, 10); av_mm(2, 11)
            for h in range(4):
                av_fin(2, h)
            sc_unit(3, 1)
            for u in range(4):
                po_unit(2, u)
            sc_unit(3, 2)
            for u in range(4, 8):
                po_unit(2, u)
            rs_unit(2)
            sc_unit(3, 3)
            # wave 3 AV, interleaved with its own scores (Act-paced)
            av_open(3)
            for ki in range(4):
                av_mm(3, ki)
            sc_unit(3, 4)
            av_mm(3, 4)
            sc_unit(3, 5)
            av_mm(3, 5)
            sc_unit(3, 6)
            av_mm(3, 6)
            sc_unit(3, 7)
            av_mm(3, 7)
            sc_unit(3, 8)
            av_mm(3, 8)
            sc_unit(3, 9)
            av_mm(3, 9)
            sc_unit(3, 10)
            av_mm(3, 10)
            sc_unit(3, 11)
            av_mm(3, 11)
            sc_unit(3, 12)
            av_mm(3, 12)
            sc_unit(3, 13)
            av_mm(3, 13)
            sc_unit(3, 14)
            av_mm(3, 14)
            sc_unit(3, 15)
            av_mm(3, 15)
            for h in range(4):
                av_fin(3, h)
            for u in range(8):
                po_unit(3, u)
            rs_unit(3)
            # final output DMAs (after all RS so the SP queue never head-blocks)
            for w in range(4):
                nc.sync.dma_start(out[128 * w : 128 * (w + 1), :], rs_out[w][:])
            psC.release()
            psS.release()

    nc.compile()
    return nc


def _prep_inputs(query, key_, value, w_q, b_q, w_k, b_k, w_v, b_v, w_o, b_o):
    """Build the 8 per-core input maps (host-side sharding / fp8 packing)."""
    import ml_dtypes

    f32 = np.float32
    bf16 = ml_dtypes.bfloat16
    e4 = ml_dtypes.float8_e4m3

    def fp8_pair(a):
        hi = a.astype(e4)
        lo = (a - hi.astype(f32)).astype(e4)
        return hi, lo

    def pack_w(wT_scaled):
        # [1024, 256] -> [p, hl, kcp, t, feat] = [128, 2, 4, 2, 256]
        hi, lo = fp8_pair(wT_scaled)
        out = np.empty((128, 2, 4, 2, 256), e4)
        for i, a in enumerate((hi, lo)):
            out[:, i] = a.reshape(4, 2, 128, 256).transpose(2, 0, 1, 3)
        return out

    def pack_x(xT):
        # [1024, 2048] -> [p, hl, blk, kcp, t, c] = [128, 2, 4, 4, 2, 512]
        hi, lo = fp8_pair(xT)
        out = np.empty((128, 2, 4, 4, 2, 512), e4)
        for i, a in enumerate((hi, lo)):
            # [kcp, t, p, blk, c] -> [p, blk, kcp, t, c]
            out[:, i] = a.reshape(4, 2, 128, 4, 512).transpose(2, 3, 0, 1, 4)
        return out

    r = np.arange(128)
    mask = (r[None, :] >= r[:, None]).astype(f32)  # [kpos, q] allowed: q >= k
    mask4 = np.ascontiguousarray(np.tile(mask, (1, 4)).astype(bf16))

    wqT = np.asarray(w_q, f32).T * SCL
    wkT = np.asarray(w_k, f32).T * SCL
    wvT = np.asarray(w_v, f32).T * SCL
    woT = np.asarray(w_o, f32).T

    xP = {}
    for g in range(B):
        for nm, src in (("q", query), ("k", key_), ("v", value)):
            xT = np.ascontiguousarray(np.asarray(src[g], f32).T)
            xP[(nm, g)] = pack_x(xT)

    in_maps = []
    for c in range(N_CORES):
        g, p = c // 4, c % 4
        fsel = slice(FPC * p, FPC * (p + 1))
        woc = (
            np.ascontiguousarray(
                woT[fsel, :].reshape(2, 128, D).transpose(1, 0, 2)
            ).astype(bf16)
        )
        bq_c = (np.asarray(b_q, f32)[fsel] * SCL).reshape(2, 128).T
        bk_c = (np.asarray(b_k, f32)[fsel] * SCL).reshape(2, 128).T
        bvt = np.broadcast_to(np.asarray(b_v, f32)[fsel] * SCL, (128, FPC))
        cF_arr = np.concatenate([bq_c, bk_c, bvt], axis=1)
        in_maps.append(
            {
                "cWQ": pack_w(wqT[:, fsel]),
                "cWK": pack_w(wkT[:, fsel]),
                "cWV": pack_w(wvT[:, fsel]),
                "cWO": woc,
                "cMASK": mask4,
                "cF": np.ascontiguousarray(cF_arr.astype(f32)),
                "xq8": xP[("q", g)],
                "xk8": xP[("k", g)],
                "xv8": xP[("v", g)],
            }
        )
    return in_maps


def run(inputs, trace=False):
    from concourse.bass_utils import run_bass_kernel_spmd

    if "nc" not in _CACHE:
        _CACHE["nc"] = _build_nc()
    nc = _CACHE["nc"]
    in_maps = _prep_inputs(
        inputs["query"], inputs["key_"], inputs["value"],
        inputs["w_q"], inputs["b_q"], inputs["w_k"], inputs["b_k"],
        inputs["w_v"], inputs["b_v"], inputs["w_o"], inputs["b_o"],
    )
    res = run_bass_kernel_spmd(
        nc, in_maps, core_ids=list(range(N_CORES)), trace=trace,
    )
    bo = np.asarray(inputs["b_o"], np.float32)
    out = np.empty((B, S, D), np.float32)
    for c in range(N_CORES):
        g, p = c // 4, c % 4
        # RS for wave w scatters q rows [512w + 128p, 512w + 128(p+1))
        core_out = np.asarray(res.results[c]["out"], np.float32)
        for w in range(4):
            out[g, 512 * w + 128 * p : 512 * w + 128 * (p + 1), :] = (
                core_out[128 * w : 128 * (w + 1)] + bo
            )
    return out, res


def kernel(**inputs):
    out, _ = run(inputs, trace=False)
    return out
